# revision 1
# baseline (speedup 1.0000x reference)
"""Data-parallel Trainium kernel for the D2DUN dense-transformer block.

Sharding: pure batch data-parallel across the 8 NeuronCores (1 sample per
core), params replicated — no cross-sample interaction anywhere in the block.
Inputs arrive FULL; we shard on batch dim 0, run the per-sample block on each
core, and the gathered full-batch outputs are returned.
"""

import numpy as np
import jax
import jax.numpy as jnp
from jax import lax

N_CORES = 8

# ---------- per-sample block (channels-last-free jax ops) ----------


def _conv(x, W, b=None, pad=0, groups=1):
    y = lax.conv_general_dilated(x, W, (1, 1), [(pad, pad), (pad, pad)],
                                 feature_group_count=groups,
                                 dimension_numbers=('NCHW', 'OIHW', 'NCHW'))
    if b is not None:
        y = y + b[None, :, None, None]
    return y


def _ln(x, w, b):
    mu = jnp.mean(x, axis=1, keepdims=True)
    var = jnp.mean((x - mu) ** 2, axis=1, keepdims=True)
    return (x - mu) * lax.rsqrt(var + 1e-5) * w[None, :, None, None] + b[None, :, None, None]


def _l2n(x):
    n = jnp.sqrt(jnp.sum(x * x, axis=-1, keepdims=True))
    return x / jnp.maximum(n, 1e-12)


def _heads(t, b, c, nh):
    return t.reshape(b, c, nh, -1).transpose(0, 2, 1, 3)


def _gelu(x):
    return jax.nn.gelu(x, approximate=False)


def _atten(p, pre, cur):
    b, c, h, w = pre.shape
    pre_ln = _ln(pre, p['n1_w'], p['n1_b'])
    cur_ln = _ln(cur, p['n2_w'], p['n2_b'])
    qv1 = _conv(_conv(cur_ln, p['qv1_w1'], p['qv1_b1']), p['qv1_w2'], p['qv1_b2'], pad=1, groups=2 * c)
    q, v1 = jnp.split(qv1, 2, axis=1)
    kv = _conv(_conv(pre_ln, p['kv_w1'], p['kv_b1']), p['kv_w2'], p['kv_b2'], pad=1, groups=2 * c)
    k, v2 = jnp.split(kv, 2, axis=1)
    nh = 4
    qh = _l2n(_heads(q.reshape(b, c, -1), b, c, nh))
    kh = _l2n(_heads(k.reshape(b, c, -1), b, c, nh))
    v1h = _heads(v1.reshape(b, c, -1), b, c, nh)
    v2h = _heads(v2.reshape(b, c, -1), b, c, nh)
    att = jax.nn.softmax(jnp.einsum('bncd,bnkd->bnck', qh, kh), axis=-1)
    v = p['w1'] * v1h + p['w2'] * v2h
    out = jnp.einsum('bnck,bnkd->bncd', att, v)
    out = out.transpose(0, 2, 1, 3).reshape(b, c, h, w)
    return _conv(out, p['out_w'], p['out_b']) + cur


def _nonlo(p, x, z):
    b, _, h, w = x.shape
    c = 31
    x0 = _ln(x, p['nx_w'], p['nx_b'])
    z0 = _ln(z, p['nz_w'], p['nz_b'])
    z1 = _conv(_conv(z0, p['t_w1'], p['t_b1']), p['t_w2'], p['t_b2'], pad=1, groups=c)
    x1 = _conv(_conv(x0, p['p_w1'], p['p_b1']), p['p_w2'], p['p_b2'], pad=1, groups=c)
    x2 = _conv(_conv(x0, p['g1_w1'], p['g1_b1']), p['g1_w2'], p['g1_b2'], pad=1, groups=c)
    z2 = _conv(_conv(z0, p['g2_w1'], p['g2_b1']), p['g2_w2'], p['g2_b2'], pad=1, groups=c)
    nh = 4
    x1h = _l2n(_heads(x1.reshape(b, c, -1), b, c, nh))
    z1h = _l2n(_heads(z1.reshape(b, c, -1), b, c, nh))
    xvh = _heads(x2.reshape(b, c, -1), b, c, nh)
    zvh = _heads(z2.reshape(b, c, -1), b, c, nh)
    att = jax.nn.softmax(jnp.einsum('bncd,bnkd->bnck', z1h, x1h), axis=-1)
    v = p['w3'] * zvh + p['w4'] * xvh
    out = jnp.einsum('bnck,bnkd->bncd', att, v)
    out = out.reshape(b, c, h, w)
    pos = _conv(_gelu(_conv(z2, p['pe_w1'], pad=1, groups=c)), p['pe_w2'], pad=1, groups=c)
    out = _conv(out, p['w_w'], p['w_b']) + pos + z
    return _conv(jnp.concatenate([x, out], axis=1), p['v_w'], p['v_b'])


def _ffn(x, w1, w2, w3):
    y = _gelu(_conv(x, w1))
    y = _gelu(_conv(y, w2, pad=1, groups=128))
    return _conv(y, w3)


def _block(x, z_pre, z_cur, PhiTb, p):
    x_exp = _conv(x, p['ce_w'], p['ce_b'], pad=1)
    z = _atten(p['atten'], z_pre, z_cur)
    phitb_exp = _conv(PhiTb, p['ce_w'], p['ce_b'], pad=1)
    x_grad = x_exp + p['lambda_step'] * (phitb_exp - x_exp)
    g = _conv(jax.nn.relu(_conv(x_grad, p['gm_w1'], p['gm_b1'], pad=1)), p['gm_w2'], p['gm_b2'], pad=1)
    x_input = x_grad + g
    x_input = _nonlo(p['nonlo'], x_input, z)
    xn = _ln(x_input, p['n1_w'], p['n1_b'])
    x_fwd = _ffn(xn, p['cf_w1'], p['cf_w2'], p['cf_w3']) + x_input
    xn2 = _ln(x_fwd, p['n2_w'], p['n2_b'])
    x_bwd = _ffn(xn2, p['cb_w1'], p['cb_w2'], p['cb_w3']) + x_fwd
    x_pred_expanded = x_input + x_bwd
    x_pred = _conv(x_pred_expanded, p['cc_w'], p['cc_b'], pad=1)
    z_out = x_pred_expanded[:, :31]
    return x_pred, z_out


# ---------- 8-core data-parallel wrapper ----------

_pmapped = None


def _get_pmapped():
    global _pmapped
    if _pmapped is None:
        _pmapped = jax.pmap(
            _block,
            axis_name='b',
            in_axes=(0, 0, 0, 0, None),
            devices=jax.devices()[:N_CORES],
        )
    return _pmapped


def kernel(x, z_pre, z_cur, PhiTb, params):
    x = np.asarray(x)
    z_pre = np.asarray(z_pre)
    z_cur = np.asarray(z_cur)
    PhiTb = np.asarray(PhiTb)
    p = jax.tree.map(jnp.asarray, params)

    B = x.shape[0]
    assert B == N_CORES, f"expected batch {N_CORES}, got {B}"
    # shard: one sample per core, keep a singleton batch dim per shard
    xs = x.reshape(N_CORES, 1, *x.shape[1:])
    zps = z_pre.reshape(N_CORES, 1, *z_pre.shape[1:])
    zcs = z_cur.reshape(N_CORES, 1, *z_cur.shape[1:])
    pts = PhiTb.reshape(N_CORES, 1, *PhiTb.shape[1:])

    fn = _get_pmapped()
    x_pred, z_out = fn(xs, zps, zcs, pts, p)
    x_pred = np.asarray(x_pred).reshape(B, *x_pred.shape[2:])
    z_out = np.asarray(z_out).reshape(B, *z_out.shape[2:])
    return x_pred.astype(np.float32), z_out.astype(np.float32)
